# revision 19
# baseline (speedup 1.0000x reference)
"""Trainium2 Bass kernel for nn_ClusteringLayer (vq_codebook, Student-t cluster assignment).

Computes, for x [65536, 512] and centroids [512, 512]:
    d2 = ||x||^2 + ||c||^2 - 2 x @ c^T          # [N, K] squared distances
    q  = 1 / (1 + d2); q = q / q.sum(axis=1)    # row-normalized Student-t kernel

Sharding: data-parallel over the N axis across 8 NeuronCores (8192 rows each),
centroids replicated. No collectives needed.

Per-core device pipeline, per [128, 512] output tile:
  PE   : psum = (-2 C^T) contraction over D in 4 bf16 matmuls
         + one augmented K=2 matmul adding (1 + ||c||^2) as bf16 coarse+residual rows
  DVE  : x_sq via fused square+row-reduce (tensor_tensor_reduce, init 1.0)
         qu = 1/t and s = sum(qu) via fused scalar_tensor_tensor divide + accum
         rs = 1/s via reciprocal_approx_fast
  ACT  : t = psum + x_sq (Identity activation with per-partition bias, PSUM evict)
  DVE  : out = qu * rs, cast to fp16
x is fed in both layouts (transposed bf16 for matmul weights, normal bf16 for x_sq).
"""

import numpy as np
from contextlib import ExitStack

try:
    from concourse import bacc, bass, tile, mybir
except ImportError:  # container layout: concourse lives in /opt/trn_rl_repo
    import sys

    sys.path.insert(0, "/opt/trn_rl_repo")
    from concourse import bacc, bass, tile, mybir

from concourse.bass_utils import run_bass_kernel_spmd
import ml_dtypes

P = 128
D = 512  # feature dim
KC = 512  # number of centroids
NCORES = 8
N_FULL = 65536
N_SHARD = N_FULL // NCORES  # 8192
BLK = 512  # x rows per DMA block (4 output tiles)
NCH = D // P  # 4 contraction chunks

F32 = mybir.dt.float32
BF16 = mybir.dt.bfloat16
F16 = mybir.dt.float16


def _act_unsafe(nc, out, in_, func, bias=0.0, scale=1.0, accum_out=None):
    """activation() without the Reciprocal accuracy guard (validated on HW for
    this kernel's argument range ~[0.5, 1.8])."""
    se = nc.scalar
    ins_ = [se.lower_ap(in_)]
    for arg in (bias, scale, 0.0):
        if isinstance(arg, bass.AP):
            ins_.append(se.lower_ap(arg))
        else:
            ins_.append(mybir.ImmediateValue(dtype=mybir.dt.float32, value=float(arg)))
    outs_ = [se.lower_ap(out)]
    if accum_out is not None:
        outs_.append(se.lower_ap(accum_out))
    return se.add_instruction(
        mybir.InstActivation(
            name=nc.get_next_instruction_name(), func=func, ins=ins_, outs=outs_
        )
    )


def build_nc(
    n_rows=N_SHARD,
    repeat=1,
    epilogue="act_recip",
    enable_asserts=False,
    ablate=(),
    xn_dtype="fp8",
    host_w=True,
):
    """Build + compile the SPMD Bass module for one core's shard of n_rows.

    ablate: perf-experiment switches (break numerics, timing only):
      'act'  — skip ACT evict, feed stt from psum directly
      'dve1' — skip the x_sq square+reduce op
      'gp'   — skip gpsimd normalize, DMA out the fp32 qu tile's first half
      'aug'  — skip the augmented c_sq matmul
      'xn'   — skip the xn DMA loads
    """
    ablate = set(ablate)
    if "xn" in ablate:
        ablate.add("dve1")  # nothing may read the unloaded xnb tile
    assert n_rows % BLK == 0
    nblk = n_rows // BLK

    nc = bacc.Bacc(
        "TRN2",
        target_bir_lowering=False,
        debug=False,
        enable_asserts=enable_asserts,
        num_devices=NCORES,
    )
    XND = {"fp8": mybir.dt.float8e4, "bf16": BF16}[xn_dtype]
    xt = nc.dram_tensor("xt", [D, n_rows], BF16, kind="ExternalInput").ap()
    if host_w:
        xw = nc.dram_tensor("xw", [P, n_rows // P], F32, kind="ExternalInput").ap()
    else:
        xn = nc.dram_tensor("xn", [n_rows, D], XND, kind="ExternalInput").ap()
    ct = nc.dram_tensor("ct", [D, KC], F32, kind="ExternalInput").ap()
    q = nc.dram_tensor("q", [n_rows, KC], F16, kind="ExternalOutput").ap()

    MUL = mybir.AluOpType.mult
    ADD = mybir.AluOpType.add
    DIV = mybir.AluOpType.divide
    IDENT = mybir.ActivationFunctionType.Identity

    with tile.TileContext(nc) as tc, ExitStack() as ctx:
        const = ctx.enter_context(tc.tile_pool(name="const", bufs=1))
        prolog = ctx.enter_context(tc.tile_pool(name="prolog", bufs=1))
        sq_pool = ctx.enter_context(tc.tile_pool(name="sqp", bufs=2))
        psum_pool = ctx.enter_context(tc.tile_pool(name="psum", bufs=7, space="PSUM"))
        xt_pool = ctx.enter_context(tc.tile_pool(name="xtp", bufs=3))
        xn_pool = ctx.enter_context(tc.tile_pool(name="xnp", bufs=3))
        t_pool = ctx.enter_context(tc.tile_pool(name="tp", bufs=5))
        qu_pool = ctx.enter_context(tc.tile_pool(name="qup", bufs=5))
        scr_pool = ctx.enter_context(tc.tile_pool(name="scr", bufs=4))
        stat_pool = ctx.enter_context(tc.tile_pool(name="stat", bufs=10))
        out_pool = ctx.enter_context(tc.tile_pool(name="outp", bufs=3))

        # ---------------- prologue: centroid prep (one-time) ----------------
        ones_col = const.tile([P, 1], F32)
        nc.vector.memset(ones_col[:], 1.0)
        ones_aug = const.tile([2, P], BF16)
        nc.vector.memset(ones_aug[:], 1.0)
        ctf = prolog.tile([P, NCH, KC], F32)  # C^T fp32, chunked on partitions
        ctb = const.tile([P, NCH, KC], BF16)  # -2 C^T bf16 (matmul rhs)
        psum_csq = psum_pool.tile([1, KC], F32, bufs=1)
        for c in range(NCH):
            nc.sync.dma_start(ctf[:, c, :], ct[c * P : (c + 1) * P, :])
        for c in range(NCH):
            nc.vector.tensor_scalar_mul(ctb[:, c, :], ctf[:, c, :], -2.0)
            sq = sq_pool.tile([P, KC], F32)
            if c % 2 == 0:
                nc.scalar.square(sq[:], ctf[:, c, :])
            else:
                nc.vector.tensor_mul(sq[:], ctf[:, c, :], ctf[:, c, :])
            # c_sq[k] = sum_d C^T[d,k]^2, accumulated over the 4 chunks
            nc.tensor.matmul(
                psum_csq[:], ones_col[:], sq[:], start=(c == 0), stop=(c == NCH - 1)
            )
        # c_sq as bf16 coarse + residual rows (augmented-matmul rhs);
        # the +1 rides in A = 1 + x_sq on the other side
        csq1 = prolog.tile([1, KC], F32)
        nc.scalar.activation(csq1[:], psum_csq[:], IDENT, bias=0.0, scale=1.0)
        csq_coarse = prolog.tile([1, KC], BF16)
        nc.vector.tensor_copy(csq_coarse[:], csq1[:])
        csq_resid = prolog.tile([1, KC], BF16)
        nc.vector.tensor_sub(csq_resid[:], csq1[:], csq_coarse[:])
        csq_aug = const.tile([2, KC], BF16)
        nc.sync.dma_start(csq_aug[0:1, :], csq_coarse[:])
        nc.sync.dma_start(csq_aug[1:2, :], csq_resid[:])
        if host_w:
            xw_t = const.tile([P, n_rows // P], F32)
            nc.sync.dma_start(xw_t[:], xw[:])

        # ---------------- main loop ----------------
        for _ in range(repeat):
            for b in range(nblk):
                xtb = xt_pool.tile([P, NCH, BLK], BF16)
                nc.sync.dma_start(
                    xtb[:],
                    xt[:, b * BLK : (b + 1) * BLK].rearrange("(c p) m -> p c m", p=P),
                )
                xnb = xn_pool.tile([P, BLK // P, D], XND)
                if not host_w and "xn" not in ablate:
                    nc.sync.dma_start(
                        xnb[:],
                        xn[b * BLK : (b + 1) * BLK, :].rearrange(
                            "(j p) d -> p j d", p=P
                        ),
                    )
                ob = out_pool.tile([P, BLK // P, KC], F16)
                for j in range(BLK // P):
                    r0 = b * BLK + j * P
                    # A = 1 + ||x||^2 per row: fused square + row-sum
                    # (affine_mul_reduce; tensor_tensor_reduce wedges the HW)
                    xsq = stat_pool.tile([P, 1], F32)
                    a1p = stat_pool.tile([P, 1], F32)
                    if host_w:
                        pass
                    elif "dve1" not in ablate:
                        sqscr = scr_pool.tile([P, D], BF16)
                        nc.vector.affine_mul_reduce(
                            sqscr[:], xsq[:], xnb[:, j, :], xnb[:, j, :],
                            scale=1.0, bias=0.0,
                        )
                        nc.vector.tensor_scalar_add(a1p[:], xsq[:], 1.0)
                    else:
                        nc.vector.memset(a1p[:], 1.0)
                    # psum = -2 x C^T + c_sq
                    ps = psum_pool.tile([P, KC], F32)
                    for c in range(NCH):
                        nc.tensor.matmul(
                            ps[:],
                            xtb[:, c, j * P : (j + 1) * P],
                            ctb[:, c, :],
                            start=(c == 0),
                            stop=(c == NCH - 1 and "aug" in ablate),
                        )
                    if "aug" not in ablate:
                        nc.tensor.matmul(
                            ps[:], ones_aug[:], csq_aug[:], start=False, stop=True
                        )
                    qu = qu_pool.tile([P, KC], F32)
                    s = stat_pool.tile([P, 1], F32)
                    if "epi" in ablate:
                        nc.scalar.activation(ob[:, j, :], ps[:], IDENT, bias=0.0, scale=1.0)
                        continue
                    if epilogue == "act_recip" and "act" not in ablate:
                        # w = 1/A; u = Recip(psum*w + 1) = A/(1 + d2) = A*q_u.
                        # The per-row factor A cancels in the normalization.
                        if host_w:
                            w = xw_t[:, b * (BLK // P) + j : b * (BLK // P) + j + 1]
                        else:
                            w = stat_pool.tile([P, 1], F32)
                            nc.vector.reciprocal_approx_fast(w[:], a1p[:])
                        _act_unsafe(
                            nc,
                            qu[:],
                            ps[:],
                            mybir.ActivationFunctionType.Reciprocal,
                            bias=1.0,
                            scale=w if host_w else w[:],
                            accum_out=s[:],
                        )
                    else:
                        # fallback: ACT evict with bias, DVE reciprocal + reduce
                        t = t_pool.tile([P, KC], F32)
                        if "act" not in ablate:
                            nc.scalar.activation(
                                t[:], ps[:], IDENT, bias=a1p[:], scale=1.0
                            )
                        else:
                            t = ps
                        nc.vector.reciprocal_approx_fast(qu[:], t[:])
                        nc.vector.reduce_sum(s[:], qu[:], axis=mybir.AxisListType.X)
                    rs = stat_pool.tile([P, 1], F32)
                    nc.vector.reciprocal_approx_fast(rs[:], s[:])
                    # out = qu * rs (row normalize), fp16 (DVE: gpsimd is ~15x
                    # slower for this op and wrecks the whole pipeline)
                    if "gp" in ablate:
                        nc.gpsimd.tensor_scalar_mul(ob[:, j, :], qu[:], rs[:])
                    else:
                        nc.vector.tensor_scalar_mul(ob[:, j, :], qu[:], rs[:])
                nc.sync.dma_start(
                    q[b * BLK : (b + 1) * BLK, :].rearrange("(j p) k -> p j k", p=P),
                    ob[:],
                )

    nc.compile()
    return nc


_NC_CACHE = {}


def _get_nc(**kw):
    key = tuple(sorted(kw.items()))
    if key not in _NC_CACHE:
        _NC_CACHE[key] = build_nc(**kw)
    return _NC_CACHE[key]


def prep_inputs(x, centroids, xn_dtype="fp8", host_w=True):
    """Host-side layout prep + per-core sharding."""
    xf = np.ascontiguousarray(np.asarray(x, dtype=np.float32))
    xb = xf.astype(ml_dtypes.bfloat16)
    xbT = np.ascontiguousarray(xb.T)  # [D, N] bf16
    ctf = np.ascontiguousarray(np.asarray(centroids, dtype=np.float32).T)  # [D, K] f32
    n = xb.shape[0]
    ns = n // NCORES
    if host_w:
        w = 1.0 / (1.0 + (xf * xf).sum(1))  # [N] f32
    else:
        xnd = {"fp8": ml_dtypes.float8_e4m3, "bf16": ml_dtypes.bfloat16}[xn_dtype]
        xn = xf.astype(xnd)
    in_maps = []
    for c in range(NCORES):
        m = {
            "xt": np.ascontiguousarray(xbT[:, c * ns : (c + 1) * ns]),
            "ct": ctf,
        }
        if host_w:
            m["xw"] = np.ascontiguousarray(
                w[c * ns : (c + 1) * ns].reshape(ns // P, P).T
            )
        else:
            m["xn"] = np.ascontiguousarray(xn[c * ns : (c + 1) * ns, :])
        in_maps.append(m)
    return in_maps


def kernel(x, centroids):
    nc = _get_nc()
    in_maps = prep_inputs(x, centroids)
    res = run_bass_kernel_spmd(nc, in_maps, core_ids=list(range(NCORES)))
    out = np.concatenate([res.results[c]["q"] for c in range(NCORES)], axis=0)
    return out.astype(np.float32)


if __name__ == "__main__":
    # smoke test with random data (no reference available standalone)
    rng = np.random.default_rng(0)
    x = rng.standard_normal((N_FULL, D), dtype=np.float32)
    c = rng.standard_normal((KC, D), dtype=np.float32)
    q = kernel(x, c)
    print("q", q.shape, q.dtype, q.sum(axis=1)[:4])
